# revision 2
# baseline (speedup 1.0000x reference)
"""BSplineWarp Trainium2 kernel.

The reference computes:
  up     = bicubic_resize(displacements, 1024, 1024)        # [N, 2, H, W]
  deltas = grid_pull_cubic(up, identity_grid)               # cubic B-spline sample
  out    = image_coordinates + moveaxis(deltas, 1, -1)

Because the sampling grid is the integer identity grid, the fractional part of
every sample coordinate is 0, so the cubic B-spline weights collapse to the
constant 3-tap stencil [1/6, 4/6, 1/6] per axis (replicate border).  Both the
bicubic upsample and that smoothing are separable linear maps along each image
axis, so the whole displacement field is exactly

  deltas[n, c] = M @ D[n, c] @ M^T,   M = S_smooth @ B_bicubic   # [1024, 32]

with M a constant [1024, 32] matrix precomputed on the host.  On device,
TT = (M @ D)^T ([64, 1024]) is built once per transform (one small matmul
pair + one PSUM->SBUF copy); each row chunk is then matmuls producing
channel-interleaved deltas, DVE adds with the streamed image_coordinates
tile, and a store.  Loads issue on the SP HWDGE ring and stores on the ACT
ring so store sem-waits never gap the load stream.

DMA granularity: each chunk covers ROWS_PP*128 image rows moved as ONE
dma_start of [128 partitions x ROWS_PP*8KB] (partition p holds rows
chunk*128*ROWS_PP + p*ROWS_PP ... +ROWS_PP-1, each partition line contiguous
in DRAM).  The row permutation this implies is folded into the host constant
M^T (its columns can be permuted freely), so compute is unchanged; bigger
transfers amortize per-DMA fixed cost toward the ~358 GB/s HBM/core limit.
Memory-bound: 33.5MB I/O per core -> ~94us roofline.

Sharding: data-parallel over the transforms axis — core i handles n in
[2i, 2i+2).  No cross-core communication.
"""

import numpy as np

N_FULL = 16
N_CORES = 8
N_PER = N_FULL // N_CORES  # transforms per core
H = W = 1024
HC = 32  # coarse control grid

ROWS_PP = 2  # image rows per SBUF partition per DMA chunk (2MB transfers)
IOBUFS = 6  # io tile pool depth

_A = -0.75  # torch bicubic coefficient


def _cubic_conv_w(t):
    offs = np.arange(-1.0, 3.0)
    d = np.abs(t[None, :] - offs[:, None])
    w_near = ((_A + 2.0) * d - (_A + 3.0)) * d * d + 1.0
    w_far = _A * (((d - 5.0) * d + 8.0) * d - 4.0)
    return np.where(d <= 1.0, w_near, np.where(d < 2.0, w_far, 0.0))


def _upsample_matrix(in_size, out_size):
    # Row o of B holds the bicubic taps: resize_last(x) == x @ B.T
    B = np.zeros((out_size, in_size))
    scale = in_size / out_size
    pos = (np.arange(out_size) + 0.5) * scale - 0.5
    i0 = np.floor(pos)
    t = pos - i0
    idx = np.clip(i0.astype(np.int64)[None, :] + np.arange(-1, 3)[:, None], 0, in_size - 1)
    w = _cubic_conv_w(t)
    for k in range(4):
        for o in range(out_size):
            B[o, idx[k, o]] += w[k, o]
    return B


def _smooth_matrix(n):
    # Cubic B-spline at integer sample points: [1/6, 4/6, 1/6], replicate clamp
    S = np.zeros((n, n))
    w = (1.0 / 6.0, 4.0 / 6.0, 1.0 / 6.0)
    for o in range(n):
        for d in (-1, 0, 1):
            S[o, min(max(o + d, 0), n - 1)] += w[d + 1]
    return S


def _row_perm(rows_pp):
    # Column order of TT matching the chunked DMA layout: position
    # chunk*(128*k) + kk*128 + p  holds image row  chunk*(128*k) + p*k + kk.
    k = rows_pp
    cr = 128 * k
    perm = np.empty(H, np.int64)
    for r in range(H // cr):
        for kk in range(k):
            for p in range(128):
                perm[r * cr + kk * 128 + p] = r * cr + p * k + kk
    return perm


def _host_matrices(rows_pp):
    M = (_smooth_matrix(H) @ _upsample_matrix(HC, H)).astype(np.float32)  # [1024, 32]
    Mt = np.ascontiguousarray(M.T[:, _row_perm(rows_pp)])  # [32, 1024], permuted
    # Channel-interleaved variant: out columns are (x, c) pairs so the second
    # matmul writes deltas already in the [..., x, c] memory order of the output.
    Mts = np.ascontiguousarray(M.T)  # [32, 1024] unpermuted (x axis)
    Mint = np.zeros((2 * HC, 2 * W), np.float32)  # [64, 2048]
    Mint[:HC, 0::2] = Mts
    Mint[HC:, 1::2] = Mts
    return Mt, Mint


_MODULE_CACHE = {}


def _build_module(reps=1, dyn_reps=1, rows_pp=None, iobufs=None):
    # reps>1 (python unroll) or dyn_reps>1 (hardware For_i loop) repeat the
    # whole body (same work, same I/O) for wall-clock benchmarking by
    # differencing; the graded path uses reps=1, dyn_reps=1.
    import concourse.bacc as bacc
    import concourse.mybir as mybir
    from concourse.tile import TileContext

    if rows_pp is None:
        rows_pp = ROWS_PP
    if iobufs is None:
        iobufs = IOBUFS

    f32 = mybir.dt.float32
    Mt, Mint = _host_matrices(rows_pp)
    k = rows_pp
    rchunks = H // (128 * k)  # chunks per image
    cw = 2 * W * k  # ct tile free size (f32 elems)

    nc = bacc.Bacc("TRN2", debug=False, num_devices=N_CORES)

    coords = nc.dram_tensor("coords", [N_PER, H, W, 2], f32, kind="ExternalInput")
    disp = nc.dram_tensor("disp", [N_PER, 2, HC, HC], f32, kind="ExternalInput")
    out = nc.dram_tensor("out", [N_PER, H, W, 2], f32, kind="ExternalOutput")
    mt_d = nc.inline_tensor(Mt, "mt_const")
    mint_d = nc.inline_tensor(Mint, "mint_const")

    coords_r = coords.ap().rearrange("n (ry p k) w c -> n ry p (k w c)", p=128, k=k)
    out_r = out.ap().rearrange("n (ry p k) w c -> n ry p (k w c)", p=128, k=k)
    disp_ap = disp.ap()

    with TileContext(nc) as tc:
        with (
            tc.tile_pool(name="const", bufs=1) as cpool,
            tc.tile_pool(name="tt", bufs=2) as ttpool,
            tc.tile_pool(name="io", bufs=iobufs) as iopool,
            tc.tile_pool(name="ptt", bufs=1, space="PSUM") as pttpool,
            tc.tile_pool(name="pd", bufs=3, space="PSUM") as pdpool,
        ):
            mt_sb = cpool.tile([HC, H], f32)
            nc.sync.dma_start(out=mt_sb[:], in_=mt_d.ap())
            mint_sb = cpool.tile([2 * HC, 2 * W], f32)
            nc.sync.dma_start(out=mint_sb[:], in_=mint_d.ap())
            # disp as [ky partitions, (n c kx)] so lhsT slices are direct
            disp_sb = cpool.tile([HC, N_PER * 2 * HC], f32)
            for n in range(N_PER):
                for c in range(2):
                    s = (n * 2 + c) * HC
                    nc.sync.dma_start(out=disp_sb[:, s : s + HC], in_=disp_ap[n, c])

            def body(n):
                # TT = (M @ D)^T for all rows at once: [64 (c,kx), 1024 y].
                # One matmul pair + one PSUM->SBUF copy per transform.
                ptt_all = pttpool.tile([2 * HC, W], f32, tag="ptt", name="ptt_all")
                s = n * 2 * HC
                for q in range(2):
                    nc.tensor.matmul(
                        ptt_all[:, q * 512 : (q + 1) * 512],
                        disp_sb[:, s : s + 2 * HC],
                        mt_sb[:, q * 512 : (q + 1) * 512],
                        start=True,
                        stop=True,
                    )
                tt_all = ttpool.tile([2 * HC, W], f32, tag="tt", name="tt_all")
                nc.scalar.copy(out=tt_all[:], in_=ptt_all[:])

                for r in range(rchunks):
                    ct = iopool.tile([128, cw], f32, tag="io", name="ct")
                    nc.sync.dma_start(out=ct[:], in_=coords_r[n, r])

                    for kk in range(k):
                        # deltas sub-rows, channel-interleaved: [128 y, 2048 (x,c)]
                        pd0 = pdpool.tile([128, 1024], f32, tag="pd", name="pd0")
                        pd1 = pdpool.tile([128, 1024], f32, tag="pd", name="pd1")
                        lhs = tt_all[:, (r * k + kk) * 128 : (r * k + kk + 1) * 128]
                        for q in range(2):
                            nc.tensor.matmul(
                                pd0[:, q * 512 : (q + 1) * 512],
                                lhs,
                                mint_sb[:, q * 512 : (q + 1) * 512],
                                start=True,
                                stop=True,
                            )
                            nc.tensor.matmul(
                                pd1[:, q * 512 : (q + 1) * 512],
                                lhs,
                                mint_sb[:, 1024 + q * 512 : 1024 + (q + 1) * 512],
                                start=True,
                                stop=True,
                            )

                        o = kk * 2 * W
                        nc.vector.tensor_add(
                            out=ct[:, o : o + 1024], in0=ct[:, o : o + 1024], in1=pd0[:]
                        )
                        nc.vector.tensor_add(
                            out=ct[:, o + 1024 : o + 2048],
                            in0=ct[:, o + 1024 : o + 2048],
                            in1=pd1[:],
                        )

                    # store on the ACT HWDGE ring so its sem waits never block
                    # load issuance on the SP ring
                    nc.scalar.dma_start(out=out_r[n, r], in_=ct[:])

            def one_rep():
                for n in range(N_PER):
                    body(n)

            if dyn_reps > 1:
                with tc.For_i(0, dyn_reps, 1):
                    one_rep()
            else:
                for _rep in range(reps):
                    one_rep()

    nc.compile()
    return nc


def _get_module(reps=1, dyn_reps=1, rows_pp=None, iobufs=None):
    if rows_pp is None:
        rows_pp = ROWS_PP
    if iobufs is None:
        iobufs = IOBUFS
    key = (reps, dyn_reps, rows_pp, iobufs)
    if key not in _MODULE_CACHE:
        _MODULE_CACHE[key] = _build_module(reps, dyn_reps, rows_pp, iobufs)
    return _MODULE_CACHE[key]


def _run(inputs, trace=False, reps=1, dyn_reps=1, **spmd_kwargs):
    from concourse import bass_utils

    nc = _get_module(reps, dyn_reps)
    coords = np.ascontiguousarray(inputs["image_coordinates"], dtype=np.float32)
    disp = np.ascontiguousarray(inputs["displacements"], dtype=np.float32)
    in_maps = [
        {
            "coords": coords[i * N_PER : (i + 1) * N_PER],
            "disp": disp[i * N_PER : (i + 1) * N_PER],
        }
        for i in range(N_CORES)
    ]
    res = bass_utils.run_bass_kernel_spmd(
        nc, in_maps, core_ids=list(range(N_CORES)), trace=trace, **spmd_kwargs
    )
    full = np.concatenate([res.results[i]["out"] for i in range(N_CORES)], axis=0)
    return full, res


def kernel(image_coordinates, displacements):
    full, _ = _run(
        {"image_coordinates": image_coordinates, "displacements": displacements}
    )
    return full


# revision 27
# speedup vs baseline: 1.6762x; 1.6762x over previous
"""BSplineWarp Trainium2 kernel.

The reference computes:
  up     = bicubic_resize(displacements, 1024, 1024)        # [N, 2, H, W]
  deltas = grid_pull_cubic(up, identity_grid)               # cubic B-spline sample
  out    = image_coordinates + moveaxis(deltas, 1, -1)

Because the sampling grid is the integer identity grid, the fractional part of
every sample coordinate is 0, so the cubic B-spline weights collapse to the
constant 3-tap stencil [1/6, 4/6, 1/6] per axis (replicate border).  Both the
bicubic upsample and that smoothing are separable linear maps along each image
axis, so the whole displacement field is exactly

  deltas[n, c] = M @ D[n, c] @ M^T,   M = S_smooth @ B_bicubic   # [1024, 32]

with M a constant [1024, 32] matrix precomputed on the host.  On device,
TT = (M @ D)^T ([64, 1024]) is built once per transform (one fp32 matmul
pair + one PSUM->SBUF copy that downcasts to bf16); each 128-row chunk is
then 4 bf16 matmuls producing channel-interleaved deltas (bf16 runs in one
PE pass where fp32 needs 2 half-speed passes — deltas are a small additive
correction so bf16's ~4e-3 relative error lands ~2e-4 on the output), fp32
DVE adds with the streamed image_coordinates tile, and a store.  Loads
issue on the SP HWDGE ring and stores on the ACT ring so store sem-waits
never gap the load stream; the startup constant loads ride the ACT ring so
the first coords load leads the SP ring.

Measured floors on this part (per-core, 8 cores active): read-only 413
GB/s, write-only 353 GB/s, concurrent read+write ~336 GB/s aggregate
regardless of burst structure (per-transfer, per-ring, and 4MB batch-phase
alternation all measure the same) — so the 33.5MB of unavoidable I/O pins
the kernel at ~100us steady state; compute is fully hidden (PE ~29us, DVE
~38us busy).  The ROWS_PP knob moves DMA granularity by folding a row
permutation into the host constant M^T (its columns can be permuted
freely); 1MB and 2MB transfers measure identical, so it stays at 1.

Sharding: data-parallel over the transforms axis — core i handles n in
[2i, 2i+2).  No cross-core communication.
"""

import numpy as np

N_FULL = 16
N_CORES = 8
N_PER = N_FULL // N_CORES  # transforms per core
H = W = 1024
HC = 32  # coarse control grid

ROWS_PP = 1  # image rows per SBUF partition per DMA chunk
IOBUFS = 8  # io tile pool depth
STORE_SPLIT = 1  # DMA stores per chunk (2 = store halves as adds complete)
SEP_OUT = 0  # 1 = adds write a separate store tile (load buffer frees at add)
RING_MODE = 0  # 1 = loads+stores share the SP ring, issue order L0,L1,S0,L2,S1,...

_A = -0.75  # torch bicubic coefficient


def _cubic_conv_w(t):
    offs = np.arange(-1.0, 3.0)
    d = np.abs(t[None, :] - offs[:, None])
    w_near = ((_A + 2.0) * d - (_A + 3.0)) * d * d + 1.0
    w_far = _A * (((d - 5.0) * d + 8.0) * d - 4.0)
    return np.where(d <= 1.0, w_near, np.where(d < 2.0, w_far, 0.0))


def _upsample_matrix(in_size, out_size):
    # Row o of B holds the bicubic taps: resize_last(x) == x @ B.T
    B = np.zeros((out_size, in_size))
    scale = in_size / out_size
    pos = (np.arange(out_size) + 0.5) * scale - 0.5
    i0 = np.floor(pos)
    t = pos - i0
    idx = np.clip(i0.astype(np.int64)[None, :] + np.arange(-1, 3)[:, None], 0, in_size - 1)
    w = _cubic_conv_w(t)
    for k in range(4):
        for o in range(out_size):
            B[o, idx[k, o]] += w[k, o]
    return B


def _smooth_matrix(n):
    # Cubic B-spline at integer sample points: [1/6, 4/6, 1/6], replicate clamp
    S = np.zeros((n, n))
    w = (1.0 / 6.0, 4.0 / 6.0, 1.0 / 6.0)
    for o in range(n):
        for d in (-1, 0, 1):
            S[o, min(max(o + d, 0), n - 1)] += w[d + 1]
    return S


def _row_perm(rows_pp):
    # Column order of TT matching the chunked DMA layout: position
    # chunk*(128*k) + kk*128 + p  holds image row  chunk*(128*k) + p*k + kk.
    k = rows_pp
    cr = 128 * k
    perm = np.empty(H, np.int64)
    for r in range(H // cr):
        for kk in range(k):
            for p in range(128):
                perm[r * cr + kk * 128 + p] = r * cr + p * k + kk
    return perm


def _host_matrices(rows_pp):
    import ml_dtypes

    M = (_smooth_matrix(H) @ _upsample_matrix(HC, H)).astype(np.float32)  # [1024, 32]
    Mt = np.ascontiguousarray(M.T[:, _row_perm(rows_pp)])  # [32, 1024], permuted
    # Channel-interleaved variant: out columns are (x, c) pairs so the second
    # matmul writes deltas already in the [..., x, c] memory order of the output.
    # bf16: the deltas matmuls run in bf16 (1 PE pass instead of fp32's 2
    # half-speed passes); deltas are a small additive correction to coords so
    # bf16's ~4e-3 relative error lands ~1e-3 on the output, well inside
    # tolerance.  The coords themselves flow fp32 end-to-end.
    Mint = np.zeros((2 * HC, 2 * W), np.float32)  # [64, 2048]
    Mint[:HC, 0::2] = M.T
    Mint[HC:, 1::2] = M.T
    return Mt, Mint.astype(ml_dtypes.bfloat16)


_MODULE_CACHE = {}


def _build_module(
    reps=1,
    dyn_reps=1,
    rows_pp=None,
    iobufs=None,
    store_split=None,
    sep_out=None,
    ring_mode=None,
):
    # reps>1 (python unroll) or dyn_reps>1 (hardware For_i loop) repeat the
    # whole body (same work, same I/O) for wall-clock benchmarking by
    # differencing; the graded path uses reps=1, dyn_reps=1.
    import concourse.bacc as bacc
    import concourse.mybir as mybir
    from concourse.tile import TileContext

    if rows_pp is None:
        rows_pp = ROWS_PP
    if iobufs is None:
        iobufs = IOBUFS
    if store_split is None:
        store_split = STORE_SPLIT
    if sep_out is None:
        sep_out = SEP_OUT
    if ring_mode is None:
        ring_mode = RING_MODE
    assert store_split == 1 or rows_pp == 1

    f32 = mybir.dt.float32
    bf16 = mybir.dt.bfloat16
    Mt, Mint = _host_matrices(rows_pp)
    k = rows_pp
    rchunks = H // (128 * k)  # chunks per image
    cw = 2 * W * k  # ct tile free size (f32 elems)

    nc = bacc.Bacc("TRN2", debug=False, num_devices=N_CORES)

    coords = nc.dram_tensor("coords", [N_PER, H, W, 2], f32, kind="ExternalInput")
    disp = nc.dram_tensor("disp", [N_PER, 2, HC, HC], f32, kind="ExternalInput")
    out = nc.dram_tensor("out", [N_PER, H, W, 2], f32, kind="ExternalOutput")
    mt_d = nc.inline_tensor(Mt, "mt_const")
    mint_d = nc.inline_tensor(Mint, "mint_const")

    coords_r = coords.ap().rearrange("n (ry p k) w c -> n ry p (k w c)", p=128, k=k)
    out_r = out.ap().rearrange("n (ry p k) w c -> n ry p (k w c)", p=128, k=k)
    disp_ap = disp.ap()

    with TileContext(nc) as tc:
        with (
            tc.tile_pool(name="const", bufs=1) as cpool,
            tc.tile_pool(name="tt", bufs=2) as ttpool,
            tc.tile_pool(name="io", bufs=iobufs) as iopool,
            tc.tile_pool(name="ot", bufs=iobufs if sep_out else 1) as opool,
            tc.tile_pool(name="ptt", bufs=1, space="PSUM") as pttpool,
            tc.tile_pool(name="pd", bufs=3, space="PSUM") as pdpool,
        ):
            # const loads ride the ACT ring (idle at start) so the first
            # coords load issues immediately on the SP ring
            mt_sb = cpool.tile([HC, H], f32)
            nc.scalar.dma_start(out=mt_sb[:], in_=mt_d.ap())
            mint_sb = cpool.tile([2 * HC, 2 * W], bf16)
            nc.scalar.dma_start(out=mint_sb[:], in_=mint_d.ap())
            # disp as [ky partitions, (n c kx)] so lhsT slices are direct
            disp_sb = cpool.tile([HC, N_PER * 2 * HC], f32)
            for n in range(N_PER):
                for c in range(2):
                    s = (n * 2 + c) * HC
                    nc.scalar.dma_start(out=disp_sb[:, s : s + HC], in_=disp_ap[n, c])

            def build_tt(n):
                # TT = (M @ D)^T for all rows at once: [64 (c,kx), 1024 y].
                # One matmul pair + one PSUM->SBUF copy per transform.
                ptt_all = pttpool.tile([2 * HC, W], f32, tag="ptt", name="ptt_all")
                s = n * 2 * HC
                for q in range(2):
                    nc.tensor.matmul(
                        ptt_all[:, q * 512 : (q + 1) * 512],
                        disp_sb[:, s : s + 2 * HC],
                        mt_sb[:, q * 512 : (q + 1) * 512],
                        start=True,
                        stop=True,
                    )
                # PSUM->SBUF copy downcasts TT to bf16 for the deltas matmuls
                tt_all = ttpool.tile([2 * HC, W], bf16, tag="tt", name="tt_all")
                nc.scalar.copy(out=tt_all[:], in_=ptt_all[:])
                return tt_all

            def compute_chunk(tt_all, n, r, ct, st):
                for kk in range(k):
                    # deltas sub-rows, channel-interleaved: [128 y, 2048 (x,c)]
                    pd0 = pdpool.tile([128, 1024], f32, tag="pd", name="pd0")
                    pd1 = pdpool.tile([128, 1024], f32, tag="pd", name="pd1")
                    lhs = tt_all[:, (r * k + kk) * 128 : (r * k + kk + 1) * 128]
                    for q in range(2):
                        nc.tensor.matmul(
                            pd0[:, q * 512 : (q + 1) * 512],
                            lhs,
                            mint_sb[:, q * 512 : (q + 1) * 512],
                            start=True,
                            stop=True,
                        )
                        nc.tensor.matmul(
                            pd1[:, q * 512 : (q + 1) * 512],
                            lhs,
                            mint_sb[:, 1024 + q * 512 : 1024 + (q + 1) * 512],
                            start=True,
                            stop=True,
                        )

                    o = kk * 2 * W
                    nc.vector.tensor_add(
                        out=st[:, o : o + 1024], in0=ct[:, o : o + 1024], in1=pd0[:]
                    )
                    if store_split == 2:
                        nc.scalar.dma_start(out=out_r[n, r][:, :1024], in_=st[:, :1024])
                    nc.vector.tensor_add(
                        out=st[:, o + 1024 : o + 2048],
                        in0=ct[:, o + 1024 : o + 2048],
                        in1=pd1[:],
                    )
                    if store_split == 2:
                        nc.scalar.dma_start(out=out_r[n, r][:, 1024:], in_=st[:, 1024:])

            def one_rep():
                if ring_mode == 1:
                    # Loads and stores share the SP HWDGE ring, issue order
                    # L0,L1,S0,L2,S1,...: the queue FIFO alternates direction
                    # in whole-1MB bursts, avoiding the packet-granularity
                    # HBM read/write mixing of the two-ring schedule.
                    prev = None
                    for n in range(N_PER):
                        tt_all = build_tt(n)
                        for r in range(rchunks):
                            ct = iopool.tile([128, cw], f32, tag="io", name="ct")
                            nc.sync.dma_start(out=ct[:], in_=coords_r[n, r])
                            if prev is not None:
                                nc.sync.dma_start(
                                    out=out_r[prev[0], prev[1]], in_=prev[2][:]
                                )
                            st = (
                                opool.tile([128, cw], f32, tag="ot", name="ot")
                                if sep_out
                                else ct
                            )
                            compute_chunk(tt_all, n, r, ct, st)
                            prev = (n, r, st)
                    nc.sync.dma_start(out=out_r[prev[0], prev[1]], in_=prev[2][:])
                    return
                for n in range(N_PER):
                    tt_all = build_tt(n)
                    for r in range(rchunks):
                        ct = iopool.tile([128, cw], f32, tag="io", name="ct")
                        nc.sync.dma_start(out=ct[:], in_=coords_r[n, r])
                        st = (
                            opool.tile([128, cw], f32, tag="ot", name="ot")
                            if sep_out
                            else ct
                        )
                        compute_chunk(tt_all, n, r, ct, st)
                        # store on the ACT HWDGE ring so its sem waits never
                        # block load issuance on the SP ring
                        if store_split == 2:
                            pass  # halves already stored inside compute_chunk
                        else:
                            nc.scalar.dma_start(out=out_r[n, r], in_=st[:])

            if dyn_reps > 1:
                with tc.For_i(0, dyn_reps, 1):
                    one_rep()
            else:
                for _rep in range(reps):
                    one_rep()

    nc.compile()
    return nc


def _get_module(
    reps=1, dyn_reps=1, rows_pp=None, iobufs=None, store_split=None, sep_out=None
):
    if rows_pp is None:
        rows_pp = ROWS_PP
    if iobufs is None:
        iobufs = IOBUFS
    if store_split is None:
        store_split = STORE_SPLIT
    if sep_out is None:
        sep_out = SEP_OUT
    key = (reps, dyn_reps, rows_pp, iobufs, store_split, sep_out)
    if key not in _MODULE_CACHE:
        _MODULE_CACHE[key] = _build_module(
            reps, dyn_reps, rows_pp, iobufs, store_split, sep_out
        )
    return _MODULE_CACHE[key]


def _run(inputs, trace=False, reps=1, dyn_reps=1, **spmd_kwargs):
    from concourse import bass_utils

    nc = _get_module(reps, dyn_reps)
    coords = np.ascontiguousarray(inputs["image_coordinates"], dtype=np.float32)
    disp = np.ascontiguousarray(inputs["displacements"], dtype=np.float32)
    in_maps = [
        {
            "coords": coords[i * N_PER : (i + 1) * N_PER],
            "disp": disp[i * N_PER : (i + 1) * N_PER],
        }
        for i in range(N_CORES)
    ]
    res = bass_utils.run_bass_kernel_spmd(
        nc, in_maps, core_ids=list(range(N_CORES)), trace=trace, **spmd_kwargs
    )
    full = np.concatenate([res.results[i]["out"] for i in range(N_CORES)], axis=0)
    return full, res


def kernel(image_coordinates, displacements):
    full, _ = _run(
        {"image_coordinates": image_coordinates, "displacements": displacements}
    )
    return full
